# revision 27
# baseline (speedup 1.0000x reference)
"""Multi-head attention (batch=2, seq=2048, dim=256, nhead=8, head_dim=256)
distributed across 8 trn2 NeuronCores.

Sharding: the 16 (batch, head) pairs are distributed 2-per-core (cores 0-3
handle batch 0 heads 0-7, cores 4-7 batch 1). Each core computes its two
heads end-to-end; the host sums the 4 partials per batch and adds the bias.

Key structure (v3):
  - Wo is folded into the v projection on the host: W' = Wo_h @ Wv_h, so the
    kernel computes v' = x @ W'^T and the AV matmul directly yields the final
    per-head output partial (bf16 to DRAM; host sums in f32).
  - AV is emitted "flipped": lhsT = E tile [sk,sq], rhs = v' [sk, o], so the
    output lands as [sq, o] — matching DRAM layout and making the softmax
    denominator a per-partition scalar at eviction time.
  - The denominator rides the AV matmul as a ones-column appended to each v'
    sk-block (rhs width 257); no DVE add-tree and no tiny densum matmuls.
  - q/k projections run fp8e4m3 DoubleRow (x and Wq/Wk shipped as fp8, with
    Wq/Wk pre-scaled by 64 on the host; the exp activation scale absorbs the
    4096x score scale). QK^T also fp8 DoubleRow as before.
  - PE-queue emission interleaves QK groups of chunk c+1 (and the j1
    projections) between the AV matmuls of chunk c so the scalar engine's
    Exp never stalls the PE; QK(j0,c0) groups ride the j0 projection phase.
  - Dummy warm-up matmuls on memset data run during the initial DMA wait so
    the PE clock (HAM) is ramped when real work arrives; inputs are split
    into <=64KB pieces across many DMA rings.
"""

import sys

if "/opt/trn_rl_repo" not in sys.path:
    sys.path.insert(0, "/opt/trn_rl_repo")

import numpy as np
import ml_dtypes

P = 128
S = 2048
D = 256
CHUNK = 512
CH = S // CHUNK  # 4 sq chunks
NKT = S // P     # 16 sk tiles
NHEAD = 8
NCORES = 8
WSCALE = 64.0
EXPSCALE = 1.0 / (16.0 * WSCALE * WSCALE)
VB = D + 1       # v' block width incl. ones column
NF8 = 16         # sk tiles whose AV runs fp8e4m3 DoubleRow (error budget)
KP = NF8 // 2    # fp8 DR pairs
NBF = NKT - NF8  # bf16 sk tiles
VB8 = 272        # 16-aligned fp8 v' block width (cols 257..271 unused)

_BUILT = None


def _build():
    import concourse.bacc as bacc
    import concourse.mybir as mybir
    import concourse.tile as tile
    from contextlib import ExitStack

    BF = mybir.dt.bfloat16
    FP8 = mybir.dt.float8e4
    F32 = mybir.dt.float32
    EXP = mybir.ActivationFunctionType.Exp
    DR = mybir.MatmulPerfMode.DoubleRow

    nc = bacc.Bacc(None, target_bir_lowering=False, debug=False)
    with tile.TileContext(nc) as tc:
        with ExitStack() as ctx:
            dram = ctx.enter_context(tc.tile_pool(name="dram", bufs=1, space="DRAM"))
            xtb_d = dram.tile([2, P, S], BF, kind="ExternalInput", name="xtb")
            xt8_d = dram.tile([P, 2 * S], FP8, kind="ExternalInput", name="xt8")
            wq8_d = dram.tile([2, P, 2 * D], FP8, kind="ExternalInput", name="wq8")
            wk8_d = dram.tile([2, P, 2 * D], FP8, kind="ExternalInput", name="wk8")
            wp_d = dram.tile([2, P, 2 * D], BF, kind="ExternalInput", name="wp")
            out_d = dram.tile([S, D], BF, kind="ExternalOutput", name="out")

            xpool = ctx.enter_context(tc.tile_pool(name="xtp", bufs=1))
            wpool = ctx.enter_context(tc.tile_pool(name="wp", bufs=1))
            xtb_sb = [xpool.tile([P, S], BF, name=f"xtb{et}") for et in range(2)]
            xt8_sb = xpool.tile([P, 2 * S], FP8, name="xt8")
            wq8_sb = [wpool.tile([P, 2 * D], FP8, name=f"wq8{j}") for j in range(2)]
            wk8_sb = [wpool.tile([P, 2 * D], FP8, name=f"wk8{j}") for j in range(2)]
            wp_sb = [wpool.tile([P, 2 * D], BF, name=f"wp{et}") for et in range(2)]
            warm_sb = wpool.tile([P, D], BF, name="warm")
            warm8_sb = wpool.tile([P, 2 * CHUNK], FP8, name="warm8")

            xt8v = xt8_sb.rearrange("p (ko s) -> p ko s", ko=2)
            wq3 = [w.rearrange("p (ko d) -> p ko d", ko=2) for w in wq8_sb]
            wk3 = [w.rearrange("p (ko d) -> p ko d", ko=2) for w in wk8_sb]

            fpool = ctx.enter_context(tc.tile_pool(name="fp", bufs=1))
            final_sb = fpool.tile([P, NKT * D], BF, name="final")

            vpool = ctx.enter_context(tc.tile_pool(name="vp", bufs=1))
            if NBF:
                v2_sb = vpool.tile([P, 2 * NBF * VB], BF, name="v2")
                v2v3 = v2_sb.rearrange("p (b c) -> p b c", c=VB)
            else:
                v2_sb = v2v3 = None
            v8_sb = vpool.tile([P, KP * 2 * 2 * VB8], FP8, name="v8")
            v8v = v8_sb.rearrange("p (t j ko c) -> p t j ko c", t=KP, j=2, ko=2)

            qkpool = ctx.enter_context(tc.tile_pool(name="qkp", bufs=2))
            epool = ctx.enter_context(tc.tile_pool(name="ep", bufs=3))
            rpool = ctx.enter_context(tc.tile_pool(name="rp", bufs=2))

            psA = ctx.enter_context(tc.tile_pool(name="psA", bufs=2, space="PSUM"))
            psB = ctx.enter_context(tc.tile_pool(name="psB", bufs=2, space="PSUM"))
            psC = ctx.enter_context(tc.tile_pool(name="psC", bufs=2, space="PSUM"))

            # ---- PE warm-up on memset data: ramps the PE clock while the
            # first input DMAs are in flight (no data dependencies)
            nc.vector.memset(warm_sb[:], 0.125)
            nc.vector.memset(warm8_sb[:], 0.125)
            w8v = warm8_sb.rearrange("p (ko s) -> p ko s", ko=2)
            for i in range(10):
                psw = psB.tile([P, CHUNK], F32, tag="psB", name="ps_warm")
                if i < 7:
                    nc.tensor.matmul(psw[:, :D], lhsT=warm_sb[:, :P], rhs=warm_sb[:],
                                     start=True, stop=True)
                else:
                    nc.tensor.matmul(psw[:], lhsT=w8v[:, :, :P], rhs=w8v[:],
                                     start=True, stop=True, perf_mode=DR)

            # ---- input DMAs. Each dma_start costs ~640ns of serial issue
            # time on its sequencer, so: scalar issues only the two critical
            # wq8 halves (it must be free for the q/k casts by ~10us); sync
            # and gpsimd carry the rest in demand order, early pieces small,
            # late pieces big.
            out_engines = [nc.sync, nc.gpsimd, nc.scalar]

            def x8p(et, c):  # xt8 chunk piece (32KB)
                o = et * S + c * CHUNK
                return (xt8_sb[:, o:o + CHUNK], xt8_d[:, o:o + CHUNK])

            def xtbp(et, c, h=None):  # xtb chunk piece (128KB or 64KB half)
                a = c * CHUNK + (0 if h is None else h * (CHUNK // 2))
                n = CHUNK if h is None else CHUNK // 2
                return (xtb_sb[et][:, a:a + n], xtb_d[et, :, a:a + n])

            def hhalf(dst, src, h):  # [P, n] half piece
                n = dst.shape[-1] // 2
                return (dst[:, h * n:(h + 1) * n], src[:, h * n:(h + 1) * n])

            sync_loads = [
                x8p(0, 0), x8p(1, 0), x8p(1, 1), x8p(1, 2), x8p(1, 3),
                xtbp(0, 0, 0), xtbp(0, 0, 1),
                hhalf(wp_sb[1][:], wp_d[1], 0), hhalf(wp_sb[1][:], wp_d[1], 1),
                xtbp(0, 1), xtbp(0, 2), xtbp(0, 3),
                (wk8_sb[1][:], wk8_d[1]),
            ]
            gps_loads = [
                hhalf(wk8_sb[0][:], wk8_d[0], 0), hhalf(wk8_sb[0][:], wk8_d[0], 1),
                x8p(0, 1), x8p(0, 2), x8p(0, 3),
                xtbp(1, 0, 0), xtbp(1, 0, 1),
                hhalf(wp_sb[0][:], wp_d[0], 0), hhalf(wp_sb[0][:], wp_d[0], 1),
                xtbp(1, 1), xtbp(1, 2), xtbp(1, 3),
                (wq8_sb[1][:], wq8_d[1]),
            ]
            scalar_loads = [
                hhalf(wq8_sb[0][:], wq8_d[0], 0), hhalf(wq8_sb[0][:], wq8_d[0], 1),
            ]
            for eng, lst in ((nc.gpsimd, gps_loads), (nc.sync, sync_loads),
                             (nc.scalar, scalar_loads)):
                for dst, srcap in lst:
                    eng.dma_start(out=dst, in_=srcap)

            # ones columns (denominator) in every v' block
            if NBF:
                nc.vector.memset(v2v3[:, :, D:VB], 1.0)
            try:
                nc.vector.memset(v8v[:, :, :, :, D:D + 1], 1.0)
            except Exception:
                for b in range(KP * 4):
                    nc.vector.memset(v8_sb[:, b * VB8 + D:b * VB8 + D + 1], 1.0)

            # ---- q/k projection, fp8 DoubleRow; casts split scalar/vector ----
            def emit_proj_chunk(j, c, qt_sb, kt_sb):
                for dst, w3, ceng in ((kt_sb, wk3[j], nc.scalar),
                                      (qt_sb, wq3[j], nc.vector)):
                    for dt in range(2):
                        ps = psB.tile([P, CHUNK], F32, tag="psB", name="ps_proj")
                        nc.tensor.matmul(
                            ps[:],
                            lhsT=w3[:, :, dt * P:(dt + 1) * P],
                            rhs=xt8v[:, :, c * CHUNK:(c + 1) * CHUNK],
                            start=True, stop=True, perf_mode=DR,
                        )
                        if ceng is nc.scalar:
                            nc.scalar.copy(
                                dst[:, dt * S + c * CHUNK: dt * S + (c + 1) * CHUNK],
                                ps[:])
                        else:
                            nc.vector.tensor_copy(
                                dst[:, dt * S + c * CHUNK: dt * S + (c + 1) * CHUNK],
                                ps[:])

            # ---- v' projection (Wo folded): one st tile, single 3D-AP cast ----
            def emit_vprime_st(st):
                ps = psB.tile([P, CHUNK], F32, tag="psB", name="ps_v")
                for et in range(2):
                    nc.tensor.matmul(
                        ps[:],
                        lhsT=xtb_sb[et][:, st * P:(st + 1) * P],
                        rhs=wp_sb[et][:],
                        start=(et == 0), stop=(et == 1),
                    )
                ps3 = ps.rearrange("p (j d) -> p j d", d=D)
                if st < NF8:
                    if st in (12, 13):
                        nc.scalar.copy(v8v[:, st // 2, :, st % 2, 0:D], ps3)
                    else:
                        nc.vector.tensor_copy(v8v[:, st // 2, :, st % 2, 0:D], ps3)
                elif st in (12, 13):
                    sb = st - NF8
                    nc.scalar.copy(v2v3[:, 2 * sb:2 * sb + 2, 0:D], ps3)
                else:
                    sb = st - NF8
                    nc.vector.tensor_copy(v2v3[:, 2 * sb:2 * sb + 2, 0:D], ps3)

            # ---- QK group: 2 DR matmuls (sk tiles 2g, 2g+1) + Exp ----
            def emit_qk_group(qt3, kt3, c, g, EE):
                E, E8v_ = EE
                ps = psA.tile([P, 2 * CHUNK], F32, tag="psA", name="ps_qk")
                for h in range(2):
                    kt_idx = 2 * g + h
                    nc.tensor.matmul(
                        ps[:, h * CHUNK:(h + 1) * CHUNK],
                        lhsT=kt3[:, :, kt_idx * P:(kt_idx + 1) * P],
                        rhs=qt3[:, :, c * CHUNK:(c + 1) * CHUNK],
                        start=True, stop=True, perf_mode=DR,
                    )
                if g < KP:
                    nc.scalar.activation(E8v_[:, :, g, :], ps[:],
                                         EXP, scale=EXPSCALE)
                else:
                    o = (2 * g - NF8) * CHUNK
                    nc.scalar.activation(E[:, o:o + 2 * CHUNK], ps[:],
                                         EXP, scale=EXPSCALE)

            # ---- AV chunk as a generator: yields after each matmul ----
            def gen_av(j, c, EE):
                E, E8v_ = EE
                for qd in range(4):
                    st = c * 4 + qd
                    ps = psC.tile([P, VB], F32, tag="psC", name="ps_av")
                    for t in range(KP):
                        nc.tensor.matmul(
                            ps[:],
                            lhsT=E8v_[:, :, t, qd * P:(qd + 1) * P],
                            rhs=v8v[:, t, j, :, 0:VB],
                            start=(t == 0), stop=(NBF == 0 and t == KP - 1),
                            perf_mode=DR,
                        )
                        yield
                    for kt_idx in range(NBF):
                        nc.tensor.matmul(
                            ps[:],
                            lhsT=E[:, kt_idx * CHUNK + qd * P: kt_idx * CHUNK + (qd + 1) * P],
                            rhs=v2_sb[:, (2 * kt_idx + j) * VB: (2 * kt_idx + j + 1) * VB],
                            start=False, stop=(kt_idx == NBF - 1),
                        )
                        yield
                    rc = rpool.tile([P, 1], F32, tag="rc", name="recip")
                    nc.vector.reciprocal(rc[:], ps[:, D:D + 1])
                    fs = final_sb[:, st * D:(st + 1) * D]
                    if j == 0:
                        nc.vector.tensor_scalar_mul(fs, ps[:, :D], rc[:])
                    else:
                        nc.vector.scalar_tensor_tensor(
                            fs, ps[:, :D], rc[:], fs,
                            op0=mybir.AluOpType.mult, op1=mybir.AluOpType.add,
                        )
                        # last chunk: 4-way DMA split to shorten the tail
                        last = c == CH - 1
                        nsplit = 4 if last else 2
                        w = D // nsplit
                        for hh in range(nsplit):
                            nout = 3 if last else 2
                            eng = out_engines[(nsplit * st + hh) % nout]
                            eng.dma_start(
                                out=out_d[st * P:(st + 1) * P, hh * w:(hh + 1) * w],
                                in_=final_sb[:, st * D + hh * w: st * D + (hh + 1) * w],
                            )

            def pair_riders(riders):
                out = []
                for i in range(0, len(riders) - 1, 2):
                    a, b = riders[i], riders[i + 1]
                    out.append(lambda a=a, b=b: (a(), b()))
                if len(riders) % 2:
                    out.append(riders[-1])
                return out

            def drive(gen, riders, positions=None):
                """Interleave rider callables between the generator's matmul
                steps — evenly, or at explicit step positions."""
                n_av = 4 * (KP + NBF)
                nr = len(riders)
                fired = 0
                for i, _ in enumerate(gen):
                    if positions is None:
                        want = ((i + 1) * nr) // n_av
                    else:
                        want = sum(1 for p in positions if p <= i + 1)
                    while fired < want:
                        riders[fired]()
                        fired += 1
                while fired < nr:
                    riders[fired]()
                    fired += 1

            # ================= emission =================
            qt_sb = [qkpool.tile([P, 2 * S], FP8, tag="qt", name=f"qt{j}")
                     for j in range(2)]
            kt_sb = [qkpool.tile([P, 2 * S], FP8, tag="kt", name=f"kt{j}")
                     for j in range(2)]
            qt3 = [t.rearrange("p (ko s) -> p ko s", ko=2) for t in qt_sb]
            kt3 = [t.rearrange("p (ko s) -> p ko s", ko=2) for t in kt_sb]

            E_tiles = {}
            QPOS = [1, 4, 7, 10, 14, 18, 22, 26]

            def make_E(j, c):
                E = (epool.tile([P, NBF * CHUNK], BF, tag="E", name=f"E_{j}_{c}")
                     if NBF else None)
                E8 = epool.tile([P, 2 * KP * CHUNK], FP8, tag="E8",
                                name=f"E8_{j}_{c}")
                E8v_ = E8.rearrange("p (ko t s) -> p ko t s", ko=2, t=KP)
                E_tiles[(j, c)] = (E, E8v_)
                return E_tiles[(j, c)]

            # P0: q/k proj head 0 with QK(j0,c0) groups interleaved once their
            # kt chunks are cast; v' tiles ride the back half
            E00 = make_E(0, 0)
            g_of = lambda g: (lambda: emit_qk_group(qt3[0], kt3[0], 0, g, E00))
            emit_proj_chunk(0, 0, qt_sb[0], kt_sb[0])
            emit_proj_chunk(0, 1, qt_sb[0], kt_sb[0])
            g_of(0)()
            emit_proj_chunk(0, 2, qt_sb[0], kt_sb[0])
            g_of(1)()
            emit_proj_chunk(0, 3, qt_sb[0], kt_sb[0])
            g_of(2)()
            vst = 0
            for g in range(3, NKT // 2):
                g_of(g)()
                nv = 2 if g < 7 else 16 - vst
                for _ in range(nv):
                    emit_vprime_st(vst)
                    vst += 1

            # j0 steady chunks: AV(c-1) with QK(c) riders
            for c in range(1, CH):
                E_new = make_E(0, c)
                riders = [
                    (lambda g=g, c=c, E=E_new: emit_qk_group(qt3[0], kt3[0], c, g, E))
                    for g in range(NKT // 2)
                ]
                drive(gen_av(0, c - 1, E_tiles[(0, c - 1)]), riders, QPOS)

            # AV(j0, 3) with riders: proj j1 chunks + QK(j1, 0) groups
            E10 = make_E(1, 0)
            riders = [
                (lambda c=c: emit_proj_chunk(1, c, qt_sb[1], kt_sb[1]))
                for c in range(CH)
            ] + [
                (lambda g=g: emit_qk_group(qt3[1], kt3[1], 0, g, E10))
                for g in range(NKT // 2)
            ]
            drive(gen_av(0, CH - 1, E_tiles[(0, CH - 1)]), riders)

            # j1 steady chunks
            for c in range(1, CH):
                E_new = make_E(1, c)
                riders = [
                    (lambda g=g, c=c, E=E_new: emit_qk_group(qt3[1], kt3[1], c, g, E))
                    for g in range(NKT // 2)
                ]
                drive(gen_av(1, c - 1, E_tiles[(1, c - 1)]), riders, QPOS)

            # final AV chunk, no riders
            drive(gen_av(1, CH - 1, E_tiles[(1, CH - 1)]), [])

    nc.compile()
    names = dict(xtb=xtb_d.name, xt8=xt8_d.name, wq8=wq8_d.name,
                 wk8=wk8_d.name, wp=wp_d.name, out=out_d.name)
    return nc, names


def _get_built():
    global _BUILT
    if _BUILT is None:
        _BUILT = _build()
    return _BUILT


def _prep_core_inputs(i, x, Wq, Wk, Wv, Wo, names):
    bf16 = ml_dtypes.bfloat16
    fp8 = ml_dtypes.float8_e4m3fn
    b = i // 4
    heads = [(2 * i) % NHEAD, (2 * i) % NHEAD + 1]

    xt = np.ascontiguousarray(x[b].T)                      # [256, 2048]
    xtb = xt.reshape(2, P, S).astype(bf16)
    xt8 = np.ascontiguousarray(
        xt.reshape(2, P, S).transpose(1, 0, 2).reshape(P, 2 * S)).astype(fp8)

    def w8_head(W, h):  # lhsT fp8 DR layout [128, (ko, d)]
        wT = W[h * D:(h + 1) * D, :].T * WSCALE            # [e, d]
        return np.ascontiguousarray(
            wT.reshape(2, P, D).transpose(1, 0, 2).reshape(P, 2 * D)).astype(fp8)

    wq8 = np.stack([w8_head(Wq, h) for h in heads])
    wk8 = np.stack([w8_head(Wk, h) for h in heads])

    def wp_head(h):  # W' = Wo_h @ Wv_h; rhs layout [et, 128, o]
        Wp = Wo[:, h * D:(h + 1) * D] @ Wv[h * D:(h + 1) * D]   # [o, e]
        return Wp.T.reshape(2, P, D)                            # [et, 128, o]

    wps = [wp_head(h) for h in heads]
    wp = np.concatenate(wps, axis=2).astype(bf16)               # [2, 128, 512]
    return {names["xtb"]: xtb, names["xt8"]: xt8, names["wq8"]: wq8,
            names["wk8"]: wk8, names["wp"]: wp}


def kernel(x, Wq, Wk, Wv, Wo, bo):
    from concourse.bass_utils import run_bass_kernel_spmd

    x = np.asarray(x, dtype=np.float32)
    Wq = np.asarray(Wq, dtype=np.float32)
    Wk = np.asarray(Wk, dtype=np.float32)
    Wv = np.asarray(Wv, dtype=np.float32)
    Wo = np.asarray(Wo, dtype=np.float32)
    bo = np.asarray(bo, dtype=np.float32)

    nc, names = _get_built()
    in_maps = [_prep_core_inputs(i, x, Wq, Wk, Wv, Wo, names) for i in range(NCORES)]
    res = run_bass_kernel_spmd(nc, in_maps, core_ids=list(range(NCORES)))

    out = np.zeros((2, S, D), dtype=np.float32)
    for b in range(2):
        acc = np.zeros((S, D), dtype=np.float32)
        for i in range(4 * b, 4 * b + 4):
            acc += res.results[i][names["out"]].astype(np.float32)
        out[b] = acc + bo[None, :]
    return out
